# revision 15
# baseline (speedup 1.0000x reference)
"""MCR2 loss kernel for 8 Trainium2 NeuronCores.

Strategy: the host sorts rows by class label (Gram matrices are invariant
to row order), splits each class evenly across the 8 cores, pads each
per-core class block to a multiple of 512 rows (4 tiles of 128), and casts
to fp8 e4m3 (products accumulate exactly in fp32 PSUM; the quantization
error largely cancels between loss_R and loss_Rc, measured 3.1e-3 vs the
f32 reference whose own rounding error is 3.6e-3).  Each core streams its
2.4 MB shard once and accumulates plain per-class Grams on the tensor
engine: for every "quad" of 4 sample tiles, one [128,128] x [128,128]
matmul (lhsT == rhs == the quad) yields the 4 per-tile 32x32 Grams on the
block diagonal of a [128,128] PSUM region; off-diagonal blocks are free
waste.  No masks, no labels, no vector-engine work on the device.

Perf structure (measured ~23.9 us vs 224.5 us baseline; fixed Tile
preamble/epilogue alone is ~12.8 us):
- per-class input DMAs alternate across the two HWDGE rings (sync/scalar),
  with class 0 split 16+44 tiles so the PE starts at the HBM round-trip;
- dummy matmuls on a zeroed tile pre-warm the PE HAM clock gate during the
  first DMA's latency, so real matmuls run at 2.4 GHz (56 ns/quad);
- accumulators are split into per-bank PSUM tiles (classes 0-3 / 4-7 / 8 /
  9) so finished groups are copied out and DMA'd to DRAM while later
  classes still accumulate - only class 9's copy+DMA chain is tail.

The host sums the diagonal blocks across quads/cores in float64 and
evaluates the 32x32 logdets.
"""

import sys

sys.path.insert(0, "/opt/trn_rl_repo")

import ml_dtypes
import numpy as np

import concourse.bacc as bacc
import concourse.mybir as mybir
import concourse.tile as tile
from concourse.bass_utils import run_bass_kernel_spmd

N, D, C = 600000, 32, 10
EPS = 0.5
NCORES = 8

_cache = {}


def _build_program(tj):
    """tj: per-class tile counts (each a multiple of 4, same on all cores)."""
    TILES = sum(tj)
    ROWS = TILES * 128
    MW = C * 128  # 1280 output cols: one [128,128] f32 region per class

    nc = bacc.Bacc(None)
    fp8 = mybir.dt.float8e4
    f32 = mybir.dt.float32
    z_dram = nc.dram_tensor("Z", [ROWS, D], fp8, kind="ExternalInput")
    out_dram = nc.dram_tensor("grams", [128, MW], f32, kind="ExternalOutput")

    # accumulator groups: classes per PSUM tile (each tile pads to its own
    # bank, so copies of a finished group never serialize against matmuls
    # still accumulating into another group)
    groups = [(0, 4), (4, 8), (8, 9), (9, 10)]

    with tile.TileContext(nc) as tc:
        with (
            tc.tile_pool(name="zin", bufs=1) as zin_pool,
            tc.tile_pool(name="warm", bufs=1) as warm_pool,
            tc.tile_pool(name="outp", bufs=1) as out_pool,
            tc.tile_pool(name="psum", bufs=1, space="PSUM") as psum_pool,
            tc.tile_pool(name="psumw", bufs=1, space="PSUM") as psumw_pool,
        ):
            accs = [
                psum_pool.tile(
                    [128, (hi - lo) * 128], f32, tag=f"acc{g}", name=f"acc{g}"
                )
                for g, (lo, hi) in enumerate(groups)
            ]
            scratch = psumw_pool.tile([128, 512], f32)
            out_sb = out_pool.tile([128, MW], f32)

            # PE pre-warm: dummy matmuls on a zeroed tile keep the PE busy
            # through the HAM activity window while the first chunks stream
            # in, so real matmuls run at 2.4 GHz from the start.
            wz = warm_pool.tile([128, 512], fp8)
            nc.gpsimd.memset(wz[:], 0)
            for _ in range(5):
                nc.tensor.matmul(
                    scratch[:], wz[:, :128], wz[:], start=True, stop=True
                )

            # one DMA per class block: rows [off, off+128*t) rearranged so
            # partition p holds rows [off + p*t, off + (p+1)*t) contiguously
            # (t*32 contiguous bytes per partition line). Alternate between
            # the two HWDGE rings so descriptor generation pipelines. The
            # first class is split so the PE can start on its first quads
            # while the bulk is still streaming.
            z_tiles = {}
            off = 0
            rings = [nc.sync, nc.scalar]
            r = 0
            for j, t in enumerate(tj):
                src = z_dram[off * 128 : (off + t) * 128, :]
                if j == 0 and t >= 20:
                    head = 16
                    za = zin_pool.tile([128, head * D], fp8, tag="z0a")
                    zb = zin_pool.tile([128, (t - head) * D], fp8, tag="z0b")
                    rings[0].dma_start(
                        za[:],
                        src[: head * 128, :].rearrange(
                            "(p t) d -> p (t d)", p=128, t=head
                        ),
                    )
                    rings[1].dma_start(
                        zb[:],
                        src[head * 128 :, :].rearrange(
                            "(p t) d -> p (t d)", p=128, t=t - head
                        ),
                    )
                    z_tiles[j] = [(za, head), (zb, t - head)]
                else:
                    z_sb = zin_pool.tile([128, t * D], fp8, tag=f"z{j}")
                    rings[r % 2].dma_start(
                        z_sb[:],
                        src.rearrange("(p t) d -> p (t d)", p=128, t=t),
                    )
                    r += 1
                    z_tiles[j] = [(z_sb, t)]
                off += t

            for g, (lo, hi) in enumerate(groups):
                acc = accs[g]
                for j in range(lo, hi):
                    t = tj[j]
                    nq = t // 4
                    q = 0
                    for z_sb, tpart in z_tiles[j]:
                        for qq in range(tpart // 4):
                            sl = z_sb[:, qq * 128 : (qq + 1) * 128]
                            nc.tensor.matmul(
                                acc[:, (j - lo) * 128 : (j - lo + 1) * 128],
                                sl,
                                sl,
                                start=(q == 0),
                                stop=(q == nq - 1),
                            )
                            q += 1
                # evacuate this group while later groups keep accumulating
                w = (hi - lo) * 128
                nc.vector.tensor_copy(
                    out_sb[:, lo * 128 : lo * 128 + w], acc[:]
                )
                nc.sync.dma_start(
                    out_dram[:, lo * 128 : lo * 128 + w],
                    out_sb[:, lo * 128 : lo * 128 + w],
                )

    nc.compile()
    return nc


def kernel(Z: np.ndarray, labels: np.ndarray) -> np.ndarray:
    Z = np.asarray(Z, dtype=np.float32)
    labels = np.asarray(labels, dtype=np.int32)
    n = Z.shape[0]

    counts = np.bincount(labels, minlength=C)
    # identical per-core capacity per class: ceil(count/8) rounded up to 4 tiles
    tj = []
    for c in counts:
        per_core = -(-int(c) // NCORES)
        t = -(-per_core // 128)
        tj.append(max(4, (t + 3) // 4 * 4))
    tj = tuple(tj)

    key = tj
    if key not in _cache:
        _cache[key] = _build_program(tj)
    nc = _cache[key]

    ROWS = sum(tj) * 128
    order = np.argsort(labels, kind="stable")
    Zb = Z.astype(ml_dtypes.float8_e4m3)
    bounds = np.concatenate([[0], np.cumsum(counts)])

    in_maps = []
    for k in range(NCORES):
        zp = np.zeros([ROWS, D], ml_dtypes.float8_e4m3)
        off = 0
        for j in range(C):
            cj = int(counts[j])
            s = k * cj // NCORES
            e = (k + 1) * cj // NCORES
            if e > s:
                zp[off : off + (e - s)] = Zb[order[bounds[j] + s : bounds[j] + e]]
            off += tj[j] * 128
        in_maps.append({"Z": zp})

    res = run_bass_kernel_spmd(nc, in_maps, core_ids=list(range(NCORES)))
    _cache["last_results"] = res

    gj = np.zeros([C, D, D], np.float64)
    for r in res.results:
        g = np.asarray(r["grams"], dtype=np.float64)
        for j in range(C):
            blk = g[:, j * 128 : (j + 1) * 128]
            for a in range(4):
                gj[j] += blk[a * 32 : (a + 1) * 32, a * 32 : (a + 1) * 32]

    g_all = gj.sum(axis=0)
    tr_pi = counts.astype(np.float64)

    nf, df = float(n), float(D)
    eye = np.eye(D)
    loss_r = 0.5 * np.linalg.slogdet(eye + (df / (nf * EPS)) * g_all)[1]
    loss_rc = 0.0
    for j in range(C):
        ld = np.linalg.slogdet(eye + (df / (tr_pi[j] * EPS)) * gj[j])[1]
        loss_rc += (tr_pi[j] / (2.0 * nf)) * ld
    loss_obj = loss_r - loss_rc
    return np.asarray([-loss_obj, loss_r, loss_rc], dtype=np.float32)
